# revision 6
# baseline (speedup 1.0000x reference)
"""MoE FFN (top-2 of 8 experts, SwiGLU) for 8 Trainium2 NeuronCores.

Strategy: expert parallelism. The router (tiny [T,H]@[H,E] matmul + softmax +
top-2) runs on host as part of sharding; tokens are dispatched ("alltoall by
routing decision") to the core owning their expert. Each core runs a dense
SwiGLU FFN over its gathered tokens in bf16 (fp32 PSUM accumulation), in a
feature-on-partition / token-on-free-dim layout so no on-device transposes are
needed and every weight byte is DMA'd exactly once, as a handful of large
contiguous transfers. The host applies the combine weights and scatter-adds
the per-expert outputs into the full output.

Per-core device program (expert e), with nht = H/128 h-tiles, f-chunks of
FCH columns (nft f-tiles each):
  g_T[f, t] = sum_i w1[h_i, f]^T @ x_T[h_i, t]        (PSUM accum over h-tiles)
  u_T[f, t] likewise with w2
  h_T[f, t] = silu(g_T + b1) * (u_T + b2)             (ACT + DVE, -> bf16)
  y_T[h, t] = sum_f w3[f, h]^T @ h_T[f, t] + b3       (PSUM accum per f-chunk,
                                                       accumulated in SBUF bf16)
w1|w2|w3 are packed into ONE per-chunk weight stream (single DMA + semaphore
per chunk; the first chunk is split into first-use-ordered pieces across both
HWDGE queues so the PE starts as soon as the first pieces land). Tokens and
the bf16 output accumulator are SBUF-resident; y leaves as one contiguous
2D DMA per token block on the scalar queue.
"""

import numpy as np
import ml_dtypes

E = 8       # experts == cores
K = 2       # top-k
H = 1024    # hidden
F = 4096    # ffn dim
BLK = 512   # max tokens per block (moving free dim of every matmul)
FCH = 512   # f-chunk size (weight streaming granularity); FCH % 128 == 0

NHT = H // 128    # h-tiles
NFCH = F // FCH   # f-chunks
NFT = FCH // 128  # f-tiles per chunk
WCOL = 3 * NFT * H  # merged w1|w2|w3 chunk columns

_BF16 = ml_dtypes.bfloat16

_kernel_cache: dict[object, object] = {}
_last_in_maps = None


def _blocks_for(max_n: int):
    """Token-block sizes covering max_n tokens: full 512-blocks plus a small
    tail block, so padded capacity hugs the real max expert load."""
    max_n = max(max_n, 16)
    nfull, rem = divmod(max_n, BLK)
    rem = (rem + 7) // 8 * 8  # keep DMA rows 16B-aligned
    sizes = [BLK] * nfull + ([rem] if rem else [])
    blocks = []
    off = 0
    for sz in sizes:
        blocks.append((off, sz))
        off += sz
    return blocks, off


def _build(blocks, use_b2: bool):
    """Build the per-core Bass/Tile program for the given token blocks."""
    import concourse.bass as bass  # noqa: F401
    import concourse.tile as tile
    from concourse import bacc, mybir

    bf16 = mybir.dt.bfloat16
    f32 = mybir.dt.float32
    AF = mybir.ActivationFunctionType

    cap = sum(sz for _, sz in blocks)

    nc = bacc.Bacc("TRN2", target_bir_lowering=False, debug=False, num_devices=E)

    # Host-side layouts are chosen so every DMA is a large 2D/3D transfer with
    # long contiguous rows (see kernel() for the packing).
    xT = nc.declare_dram_parameter("xT", [128, NHT * cap], bf16, isOutput=False)
    w123 = nc.declare_dram_parameter("w123", [NFCH, 128, WCOL], bf16, isOutput=False)
    b1 = nc.declare_dram_parameter("b1", [128, F // 128], f32, isOutput=False)
    b3 = nc.declare_dram_parameter("b3", [128, NHT], f32, isOutput=False)
    if use_b2:
        b2 = nc.declare_dram_parameter("b2", [128, F // 128], f32, isOutput=False)
    yT = nc.declare_dram_parameter("yT", [128, NHT * cap], bf16, isOutput=True)

    with tile.TileContext(nc) as tc:
        with (
            tc.tile_pool(name="xp", bufs=1) as xp,
            tc.tile_pool(name="yp", bufs=1) as yp,
            tc.tile_pool(name="wp", bufs=2) as wp,
            tc.tile_pool(name="hp", bufs=2) as hp,
            tc.tile_pool(name="sp", bufs=3) as sp,
            tc.tile_pool(name="bp", bufs=1) as bp,
            tc.tile_pool(name="pg", bufs=2, space="PSUM") as pg,
            tc.tile_pool(name="pu", bufs=3, space="PSUM") as pu,
            tc.tile_pool(name="py", bufs=3, space="PSUM") as py,
        ):
            # b1 rides first on gpsimd (2KB; needed by the first silu ~12us in)
            b1t = bp.tile([128, F // 128], f32, tag="b1", name="b1t")
            nc.gpsimd.dma_start(b1t[:], b1[:])

            # Tokens (resident, bf16): one [128, NHT*cap] tile in BLOCK-major
            # column order — token block at offset `off` occupies columns
            # [NHT*off, NHT*(off+sz)), h-tile i contiguous inside it. The host
            # supplies the identical layout, so each block is ONE contiguous
            # 2D transfer with multi-KB rows.
            xall = xp.tile([128, NHT * cap], bf16, name="xall")

            def xsl(i, off, sz):  # moving operand [128, sz] for h-tile i
                base = NHT * off + i * sz
                return xall[:, base:base + sz]

            # Output accumulator (resident, bf16), same column layout as xall.
            yall = yp.tile([128, NHT * cap], bf16, name="yall")

            def ysl(i, off, sz):
                base = NHT * off + i * sz
                return yall[:, base:base + sz]

            # Prologue, ordered by first use across THREE HWDGE queues
            # (each queue transfers FIFO at ~120GB/s early on, so critical
            # bytes are spread and interleaved in consumption order): the
            # first psg needs w1[j0] (sync) plus block-0 h-tiles (per-tile
            # pieces alternating scalar/sync, consumed in order i=0..7);
            # w2 pieces ride gpsimd; w3 chunk 0 follows there (first used
            # ~30us in). The PE starts ~11us in and trickles briefly while
            # block 0 streams.
            _, sz0 = blocks[0]
            wc = wp.tile([128, WCOL], bf16, tag="w", name="wc")
            nc.sync.dma_start(wc[:, 0:H], w123[0][:, 0:H])  # w1 piece j0
            for i in range(NHT):
                isl = slice(i * sz0, (i + 1) * sz0)
                q = nc.scalar if i % 2 == 0 else nc.sync
                q.dma_start(xall[:, isl], xT[:, isl])
            for j in range(1, NFT):
                jsl = slice(j * H, (j + 1) * H)
                nc.sync.dma_start(wc[:, jsl], w123[0][:, jsl])
            for j in range(NFT):
                j2 = slice((NFT + j) * H, (NFT + j + 1) * H)
                nc.gpsimd.dma_start(wc[:, j2], w123[0][:, j2])
            w3sl = slice(2 * NFT * H, 3 * NFT * H)
            nc.gpsimd.dma_start(wc[:, w3sl], w123[0][:, w3sl])
            b3t = bp.tile([128, NHT], f32, tag="b3", name="b3t")
            nc.gpsimd.dma_start(b3t[:], b3[:])
            if use_b2:
                b2t = bp.tile([128, F // 128], f32, tag="b2", name="b2t")
                nc.gpsimd.dma_start(b2t[:], b2[:])
            for off, sz in blocks[1:]:
                lo, hi = NHT * off, NHT * (off + sz)
                nc.scalar.dma_start(xall[:, lo:hi], xT[:, lo:hi])

            for fc in range(NFCH):
                if fc > 0:
                    # Stream this f-chunk's w1|w2|w3 as ONE transfer (one
                    # semaphore; each byte loaded once).
                    wc = wp.tile([128, WCOL], bf16, tag="w", name="wc")
                    nc.sync.dma_start(wc[:], w123[fc])

                def stage_b(off, sz, ht_tiles, wc=wc):
                    # Stage B: y_T[h, tok] += w3_chunk.T @ h_T
                    # w3 columns: (j, h) -> f-tile j, output col h.
                    for i in range(NHT):
                        psy = py.tile([128, sz], f32, tag="y", name="psy")
                        for j in range(NFT):
                            base = 2 * NFT * H + j * H + i * 128
                            nc.tensor.matmul(
                                psy[:],
                                wc[:, base:base + 128],
                                ht_tiles[j][:],
                                start=(j == 0), stop=(j == NFT - 1),
                            )
                        if fc == 0:
                            nc.scalar.activation(
                                ysl(i, off, sz), psy[:], AF.Identity,
                                bias=b3t[:, i:i + 1],
                            )
                        else:
                            nc.vector.tensor_add(
                                ysl(i, off, sz), ysl(i, off, sz), psy[:]
                            )
                    if fc == NFCH - 1:
                        # Drain y per h-tile, alternating the two idle
                        # queues, so each piece leaves as soon as its add
                        # lands and the final drain is one small piece.
                        for i2 in range(NHT):
                            lo = NHT * off + i2 * sz
                            q = nc.scalar if i2 % 2 == 0 else nc.sync
                            q.dma_start(yT[:, lo:lo + sz], yall[:, lo:lo + sz])

                pending = None  # (off, sz, ht_tiles) awaiting stage B
                for off, sz in blocks:
                    # Stage A: h_T[f, tok] = silu(g_T + b1) * (u_T + b2)
                    # w1/w2 columns: (j, i, q) -> f-tile j, h-tile i, col q.
                    ht_tiles = []
                    for j in range(NFT):
                        fg = fc * NFT + j  # global f-tile index
                        psg = pg.tile([128, sz], f32, tag="g", name="psg")
                        for i in range(NHT):
                            base = (j * NHT + i) * 128
                            nc.tensor.matmul(
                                psg[:], wc[:, base:base + 128], xsl(i, off, sz),
                                start=(i == 0), stop=(i == NHT - 1),
                            )
                        s = sp.tile([128, sz], f32, tag="s", name="stile")
                        nc.scalar.activation(
                            s[:], psg[:], AF.Silu, bias=b1t[:, fg:fg + 1]
                        )
                        psu = pu.tile([128, sz], f32, tag="u", name="psu")
                        for i in range(NHT):
                            base = (NFT + j) * H + i * 128
                            nc.tensor.matmul(
                                psu[:], wc[:, base:base + 128], xsl(i, off, sz),
                                start=(i == 0), stop=(i == NHT - 1),
                            )
                        h = hp.tile([128, sz], bf16, tag=f"h{j}", name=f"htile{j}")
                        if use_b2:
                            u2 = sp.tile([128, sz], f32, tag="u2", name="u2tile")
                            nc.scalar.activation(
                                u2[:], psu[:], AF.Identity, bias=b2t[:, fg:fg + 1]
                            )
                            nc.vector.tensor_mul(h[:], s[:], u2[:])
                        else:
                            nc.vector.tensor_mul(h[:], s[:], psu[:])
                        ht_tiles.append(h)

                    if pending is not None:
                        stage_b(*pending)
                    pending = (off, sz, ht_tiles)
                stage_b(*pending)

    nc.finalize()
    return nc


def _route(x2d: np.ndarray, router_w: np.ndarray):
    """Host router: softmax over experts, top-2. Returns per-expert token
    index lists and combine weights."""
    logits = x2d @ router_w                       # [T, E]
    logits -= logits.max(axis=-1, keepdims=True)
    p = np.exp(logits, dtype=np.float32)
    p /= p.sum(axis=-1, keepdims=True)
    # top-2 expert ids per token (ties: lower index first, like lax.top_k)
    order = np.argsort(-p, axis=-1, kind="stable")[:, :K]   # [T, K]
    idx_e, cw_e = [], []
    for e in range(E):
        sel = np.nonzero((order == e).any(axis=1))[0]
        idx_e.append(sel)
        cw_e.append(p[sel, e])
    return idx_e, cw_e


def _pack_w12(w: np.ndarray) -> np.ndarray:
    """[H, F] f32 -> [NFCH, 128, NFT*H] bf16 with column order (j, i, q):
    chunk c, partition p, f-tile j, h-tile i, col q = w[i*128+p, c*FCH+j*128+q].
    """
    t = np.asarray(w, dtype=np.float32).reshape(NHT, 128, NFCH, NFT, 128)
    t = t.transpose(2, 1, 3, 0, 4)  # [c, p, j, i, q]
    return np.ascontiguousarray(t.astype(_BF16)).reshape(NFCH, 128, NFT * H)


def _pack_w3(w: np.ndarray) -> np.ndarray:
    """[F, H] f32 -> [NFCH, 128, NFT*H] bf16 with column order (j, h):
    chunk c, partition p (= f within f-tile j) -> w[c*FCH+j*128+p, h]."""
    t = np.asarray(w, dtype=np.float32).reshape(NFCH, NFT, 128, H)
    t = t.transpose(0, 2, 1, 3)  # [c, p, j, h]
    return np.ascontiguousarray(t.astype(_BF16)).reshape(NFCH, 128, NFT * H)


def _pack_x(x2d_bf16: np.ndarray, blocks) -> np.ndarray:
    """[cap, H] bf16 -> [128, NHT*cap] block-major: block at token offset
    `off` spans cols [NHT*off, NHT*(off+sz)), h-tile i contiguous inside it:
    col = NHT*off + i*sz + t."""
    return np.ascontiguousarray(np.concatenate(
        [
            x2d_bf16[off:off + sz].reshape(sz, NHT, 128)
            .transpose(2, 1, 0).reshape(128, NHT * sz)
            for off, sz in blocks
        ],
        axis=1,
    ))


def kernel(x, router_w, w1, b1, w2, b2, w3, b3):
    from concourse.bass_utils import run_bass_kernel_spmd

    B, S, _ = x.shape
    T = B * S
    x2d = np.ascontiguousarray(x, dtype=np.float32).reshape(T, H)

    idx_e, cw_e = _route(x2d, np.asarray(router_w, dtype=np.float32))
    max_n = max(len(i) for i in idx_e)
    blocks, cap = _blocks_for(max_n)

    use_b2 = bool(np.any(b2))
    key = (tuple(blocks), use_b2)
    nc = _kernel_cache.get(key)
    if nc is None:
        nc = _build(blocks, use_b2)
        _kernel_cache[key] = nc

    in_maps = []
    for e in range(E):
        idx = idx_e[e]
        xg = np.zeros((cap, H), dtype=np.float32)
        xg[: len(idx)] = x2d[idx]
        m = {
            "xT": _pack_x(xg.astype(_BF16), blocks),
            "w123": np.ascontiguousarray(np.concatenate(
                [_pack_w12(w1[e]), _pack_w12(w2[e]), _pack_w3(w3[e])], axis=2
            )),
            "b1": np.ascontiguousarray(
                np.asarray(b1[e], dtype=np.float32).reshape(F // 128, 128).T
            ),
            "b3": np.ascontiguousarray(
                np.asarray(b3[e], dtype=np.float32).reshape(NHT, 128).T
            ),
        }
        if use_b2:
            m["b2"] = np.ascontiguousarray(
                np.asarray(b2[e], dtype=np.float32).reshape(F // 128, 128).T
            )
        in_maps.append(m)

    global _last_in_maps
    _last_in_maps = in_maps
    res = run_bass_kernel_spmd(nc, in_maps, core_ids=list(range(E)))

    out = np.zeros((T, H), dtype=np.float32)
    for e in range(E):
        idx = idx_e[e]
        n = len(idx)
        # yT [128, NHT*cap] block-major bf16 -> y[t, h]
        yTe = res.results[e]["yT"]
        ye = np.empty((n, H), dtype=np.float32)
        for off, sz in blocks:
            if off >= n:
                break
            m = min(sz, n - off)
            blk = yTe[:, NHT * off:NHT * (off + sz)].astype(np.float32)
            ye[off:off + m] = (
                blk.reshape(128, NHT, sz).transpose(2, 1, 0).reshape(sz, H)[:m]
            )
        out[idx] += ye * cw_e[e][:, None]
    return out.reshape(B, S, H)


# revision 7
# speedup vs baseline: 1.0452x; 1.0452x over previous
"""MoE FFN (top-2 of 8 experts, SwiGLU) for 8 Trainium2 NeuronCores.

Strategy: expert parallelism with overflow balancing. The router (tiny
[T,H]@[H,E] matmul + softmax + top-2) runs on host as part of sharding; tokens
are dispatched ("alltoall by routing decision") to the core owning their
expert. Each core runs a dense SwiGLU FFN over its gathered tokens in bf16
(fp32 PSUM accumulation), in a feature-on-partition / token-on-free-dim layout
so no on-device transposes are needed and every weight byte is DMA'd once per
consuming core, as a handful of large contiguous transfers. The host applies
the combine weights and scatter-adds the per-expert outputs into the output.

Load balancing: every core processes Q=2048 primary tokens of its own expert
(zero-padded for cold experts) plus ONE small overflow block of V tokens that
may belong to a DIFFERENT (hot) expert, with its own second weight stream —
so per-core capacity is Q+V (V ~ 48) instead of max_n (~2184), holding the
tensor-engine roofline to near the perfectly balanced minimum regardless of
routing skew.

Per-core device program (expert e), with nht = H/128 h-tiles, f-chunks of
FCH columns (nft f-tiles each):
  g_T[f, t] = sum_i w1[h_i, f]^T @ x_T[h_i, t]        (PSUM accum over h-tiles)
  u_T[f, t] likewise with w2
  h_T[f, t] = silu(g_T + b1) * (u_T + b2)             (ACT + DVE, -> bf16)
  y_T[h, t] = sum_f w3[f, h]^T @ h_T[f, t] + b3       (PSUM accum per f-chunk,
                                                       accumulated in SBUF bf16)
w1|w2|w3 are packed into ONE per-chunk weight stream per expert (single DMA +
semaphore per chunk; chunk 0 is split into first-use-ordered pieces across
three HWDGE queues sized so the PE starts ~13us in and never starves — each
queue moves ~130GB/s FIFO, so criticality = queue prefix bytes). Tokens and
the bf16 output accumulator are SBUF-resident; y leaves per h-tile on the two
idle queues as its final accumulate lands.
"""

import numpy as np
import ml_dtypes

E = 8       # experts == cores
K = 2       # top-k
H = 1024    # hidden
F = 4096    # ffn dim
BLK = 512   # max tokens per block (moving free dim of every matmul)
FCH = 512   # f-chunk size (weight streaming granularity); FCH % 128 == 0

NHT = H // 128    # h-tiles
NFCH = F // FCH   # f-chunks
NFT = FCH // 128  # f-tiles per chunk
WCOL = 3 * NFT * H  # merged w1|w2|w3 chunk columns

_BF16 = ml_dtypes.bfloat16

_kernel_cache: dict[object, object] = {}
_last_in_maps = None


def _blocks_for(max_n: int):
    """Token-block sizes covering max_n tokens: full 512-blocks plus a small
    tail block, so padded capacity hugs the real max expert load."""
    max_n = max(max_n, 16)
    nfull, rem = divmod(max_n, BLK)
    rem = (rem + 7) // 8 * 8  # keep DMA rows 16B-aligned
    sizes = [BLK] * nfull + ([rem] if rem else [])
    blocks = []
    off = 0
    for sz in sizes:
        blocks.append((off, sz))
        off += sz
    return blocks, off


def _plan_overflow(loads):
    """Balanced split: Q primary tokens per core plus one V-token overflow
    block holding a slice of some (hot) expert's excess. Returns
    (Q, V, slices[(expert, start, len)] per core) or None if infeasible /
    pointless."""
    Q = (sum(loads) // E) // BLK * BLK
    if Q < BLK:
        return None
    over = [max(0, n - Q) for n in loads]
    if sum(over) == 0:
        return None
    for V in range(8, BLK + 1, 8):
        if sum(-(-o // V) for o in over if o) <= E:
            slices = []
            for e, o in enumerate(over):
                s = 0
                while s < o:
                    slices.append((e, Q + s, min(V, o - s)))
                    s += V
            while len(slices) < E:
                slices.append((0, 0, 0))
            return Q, V, slices
    return None


def _build(blocks, use_b2: bool, split: bool):
    """Build the per-core Bass/Tile program. If `split`, the last block uses
    a second weight stream (w123o) so it can hold another expert's tokens."""
    import concourse.bass as bass  # noqa: F401
    import concourse.tile as tile
    from concourse import bacc, mybir

    bf16 = mybir.dt.bfloat16
    f32 = mybir.dt.float32
    AF = mybir.ActivationFunctionType

    cap = sum(sz for _, sz in blocks)

    nc = bacc.Bacc("TRN2", target_bir_lowering=False, debug=False, num_devices=E)

    xT = nc.declare_dram_parameter("xT", [128, NHT * cap], bf16, isOutput=False)
    w123 = nc.declare_dram_parameter("w123", [NFCH, 128, WCOL], bf16, isOutput=False)
    if split:
        w123o = nc.declare_dram_parameter(
            "w123o", [NFCH, 128, WCOL], bf16, isOutput=False
        )
    b1 = nc.declare_dram_parameter("b1", [128, F // 128], f32, isOutput=False)
    b3 = nc.declare_dram_parameter("b3", [128, NHT], f32, isOutput=False)
    if use_b2:
        b2 = nc.declare_dram_parameter("b2", [128, F // 128], f32, isOutput=False)
    yT = nc.declare_dram_parameter("yT", [128, NHT * cap], bf16, isOutput=True)

    with tile.TileContext(nc) as tc:
        with (
            tc.tile_pool(name="xp", bufs=1) as xp,
            tc.tile_pool(name="yp", bufs=1) as yp,
            tc.tile_pool(name="wp", bufs=2) as wp,
            tc.tile_pool(name="wo", bufs=2) as wo,
            tc.tile_pool(name="hp", bufs=2) as hp,
            tc.tile_pool(name="sp", bufs=3) as sp,
            tc.tile_pool(name="bp", bufs=1) as bp,
            tc.tile_pool(name="pg", bufs=2, space="PSUM") as pg,
            tc.tile_pool(name="pu", bufs=3, space="PSUM") as pu,
            tc.tile_pool(name="py", bufs=3, space="PSUM") as py,
        ):
            # b1 rides first on gpsimd (2KB; needed by the first silu)
            b1t = bp.tile([128, F // 128], f32, tag="b1", name="b1t")
            nc.gpsimd.dma_start(b1t[:], b1[:])

            # Tokens (resident, bf16), block-major columns: block at offset
            # `off` spans cols [NHT*off, NHT*(off+sz)), h-tile i contiguous
            # inside it (col = NHT*off + i*sz + t). Host supplies the same
            # layout, so each block is ONE contiguous 2D transfer.
            xall = xp.tile([128, NHT * cap], bf16, name="xall")

            def xsl(i, off, sz):  # moving operand [128, sz] for h-tile i
                base = NHT * off + i * sz
                return xall[:, base:base + sz]

            # Output accumulator (resident, bf16), same column layout.
            yall = yp.tile([128, NHT * cap], bf16, name="yall")

            def ysl(i, off, sz):
                base = NHT * off + i * sz
                return yall[:, base:base + sz]

            # Prologue, ordered by first use across three ~130GB/s FIFO
            # queues so the first matmul fires ~13us in and the PE never
            # waits again: block-0 thirds land everywhere by ~11us, w1
            # piece j lands (sync) just as psg j starts, w2 piece j
            # (gpsimd) just before psu j, w3+b3 and the later x blocks
            # trail on scalar/gpsimd well ahead of their first use.
            _, sz0 = blocks[0]
            wc = wp.tile([128, WCOL], bf16, tag="w", name="wc")
            nc.sync.dma_start(xall[:, 0:3 * sz0], xT[:, 0:3 * sz0])
            nc.gpsimd.dma_start(xall[:, 3 * sz0:6 * sz0], xT[:, 3 * sz0:6 * sz0])
            nc.scalar.dma_start(xall[:, 6 * sz0:8 * sz0], xT[:, 6 * sz0:8 * sz0])
            for j in range(NFT):
                jsl = slice(j * H, (j + 1) * H)
                nc.sync.dma_start(wc[:, jsl], w123[0][:, jsl])
                j2 = slice((NFT + j) * H, (NFT + j + 1) * H)
                nc.gpsimd.dma_start(wc[:, j2], w123[0][:, j2])
            if len(blocks) > 1:
                off1, sz1 = blocks[1]
                lo, hi = NHT * off1, NHT * (off1 + sz1)
                nc.scalar.dma_start(xall[:, lo:hi], xT[:, lo:hi])
            w3sl = slice(2 * NFT * H, 3 * NFT * H)
            nc.scalar.dma_start(wc[:, w3sl], w123[0][:, w3sl])
            b3t = bp.tile([128, NHT], f32, tag="b3", name="b3t")
            nc.scalar.dma_start(b3t[:], b3[:])
            if use_b2:
                b2t = bp.tile([128, F // 128], f32, tag="b2", name="b2t")
                nc.scalar.dma_start(b2t[:], b2[:])
            for off, sz in blocks[2:]:
                lo, hi = NHT * off, NHT * (off + sz)
                nc.scalar.dma_start(xall[:, lo:hi], xT[:, lo:hi])
            if split:
                wco = wo.tile([128, WCOL], bf16, tag="wo", name="wco")
                nc.gpsimd.dma_start(wco[:], w123o[0])

            for fc in range(NFCH):
                if fc > 0:
                    # Stream this f-chunk's w1|w2|w3 as ONE transfer each
                    # (one semaphore; each byte loaded once).
                    wc = wp.tile([128, WCOL], bf16, tag="w", name="wc")
                    nc.sync.dma_start(wc[:], w123[fc])
                    if split:
                        wco = wo.tile([128, WCOL], bf16, tag="wo", name="wco")
                        nc.gpsimd.dma_start(wco[:], w123o[fc])

                def stage_b(off, sz, ht_tiles, wt):
                    # Stage B: y_T[h, tok] += w3_chunk.T @ h_T
                    # w3 columns: (j, h) -> f-tile j, output col h.
                    for i in range(NHT):
                        psy = py.tile([128, sz], f32, tag="y", name="psy")
                        for j in range(NFT):
                            base = 2 * NFT * H + j * H + i * 128
                            nc.tensor.matmul(
                                psy[:],
                                wt[:, base:base + 128],
                                ht_tiles[j][:],
                                start=(j == 0), stop=(j == NFT - 1),
                            )
                        if fc == 0:
                            nc.scalar.activation(
                                ysl(i, off, sz), psy[:], AF.Identity,
                                bias=b3t[:, i:i + 1],
                            )
                        else:
                            nc.vector.tensor_add(
                                ysl(i, off, sz), ysl(i, off, sz), psy[:]
                            )
                        if fc == NFCH - 1:
                            # Drain y per h-tile on the two now-idle queues
                            # as soon as its final accumulate lands.
                            lo = NHT * off + i * sz
                            q = nc.scalar if i % 2 == 0 else nc.sync
                            q.dma_start(yT[:, lo:lo + sz], yall[:, lo:lo + sz])

                pending = None  # (off, sz, ht_tiles, wt) awaiting stage B
                for bi, (off, sz) in enumerate(blocks):
                    wt = wco if (split and bi == len(blocks) - 1) else wc
                    # Stage A: h_T[f, tok] = silu(g_T + b1) * (u_T + b2)
                    # w1/w2 columns: (j, i, q) -> f-tile j, h-tile i, col q.
                    ht_tiles = []
                    for j in range(NFT):
                        fg = fc * NFT + j  # global f-tile index
                        psg = pg.tile([128, sz], f32, tag="g", name="psg")
                        for i in range(NHT):
                            base = (j * NHT + i) * 128
                            nc.tensor.matmul(
                                psg[:], wt[:, base:base + 128], xsl(i, off, sz),
                                start=(i == 0), stop=(i == NHT - 1),
                            )
                        s = sp.tile([128, sz], f32, tag="s", name="stile")
                        nc.scalar.activation(
                            s[:], psg[:], AF.Silu, bias=b1t[:, fg:fg + 1]
                        )
                        psu = pu.tile([128, sz], f32, tag="u", name="psu")
                        for i in range(NHT):
                            base = (NFT + j) * H + i * 128
                            nc.tensor.matmul(
                                psu[:], wt[:, base:base + 128], xsl(i, off, sz),
                                start=(i == 0), stop=(i == NHT - 1),
                            )
                        h = hp.tile([128, sz], bf16, tag=f"h{j}", name=f"htile{j}")
                        if use_b2:
                            u2 = sp.tile([128, sz], f32, tag="u2", name="u2tile")
                            nc.scalar.activation(
                                u2[:], psu[:], AF.Identity, bias=b2t[:, fg:fg + 1]
                            )
                            nc.vector.tensor_mul(h[:], s[:], u2[:])
                        else:
                            nc.vector.tensor_mul(h[:], s[:], psu[:])
                        ht_tiles.append(h)

                    if pending is not None:
                        stage_b(*pending)
                    pending = (off, sz, ht_tiles, wt)
                stage_b(*pending)

    nc.finalize()
    return nc


def _route(x2d: np.ndarray, router_w: np.ndarray):
    """Host router: softmax over experts, top-2. Returns per-expert token
    index lists and combine weights."""
    logits = x2d @ router_w                       # [T, E]
    logits -= logits.max(axis=-1, keepdims=True)
    p = np.exp(logits, dtype=np.float32)
    p /= p.sum(axis=-1, keepdims=True)
    # top-2 expert ids per token (ties: lower index first, like lax.top_k)
    order = np.argsort(-p, axis=-1, kind="stable")[:, :K]   # [T, K]
    idx_e, cw_e = [], []
    for e in range(E):
        sel = np.nonzero((order == e).any(axis=1))[0]
        idx_e.append(sel)
        cw_e.append(p[sel, e])
    return idx_e, cw_e


def _pack_w12(w: np.ndarray) -> np.ndarray:
    """[H, F] f32 -> [NFCH, 128, NFT*H] bf16 with column order (j, i, q):
    chunk c, partition p, f-tile j, h-tile i, col q = w[i*128+p, c*FCH+j*128+q].
    """
    t = np.asarray(w, dtype=np.float32).reshape(NHT, 128, NFCH, NFT, 128)
    t = t.transpose(2, 1, 3, 0, 4)  # [c, p, j, i, q]
    return np.ascontiguousarray(t.astype(_BF16)).reshape(NFCH, 128, NFT * H)


def _pack_w3(w: np.ndarray) -> np.ndarray:
    """[F, H] f32 -> [NFCH, 128, NFT*H] bf16 with column order (j, h):
    chunk c, partition p (= f within f-tile j) -> w[c*FCH+j*128+p, h]."""
    t = np.asarray(w, dtype=np.float32).reshape(NFCH, NFT, 128, H)
    t = t.transpose(0, 2, 1, 3)  # [c, p, j, h]
    return np.ascontiguousarray(t.astype(_BF16)).reshape(NFCH, 128, NFT * H)


def _pack_x(x2d_bf16: np.ndarray, blocks) -> np.ndarray:
    """[cap, H] bf16 -> [128, NHT*cap] block-major (see _build.xsl)."""
    return np.ascontiguousarray(np.concatenate(
        [
            x2d_bf16[off:off + sz].reshape(sz, NHT, 128)
            .transpose(2, 1, 0).reshape(128, NHT * sz)
            for off, sz in blocks
        ],
        axis=1,
    ))


def _unpack_y(yTe, blocks, n: int) -> np.ndarray:
    """[128, NHT*cap] block-major bf16 -> [n, H] f32 (first n tokens)."""
    ye = np.empty((n, H), dtype=np.float32)
    for off, sz in blocks:
        if off >= n:
            break
        m = min(sz, n - off)
        blk = yTe[:, NHT * off:NHT * (off + sz)].astype(np.float32)
        ye[off:off + m] = (
            blk.reshape(128, NHT, sz).transpose(2, 1, 0).reshape(sz, H)[:m]
        )
    return ye


def kernel(x, router_w, w1, b1, w2, b2, w3, b3):
    from concourse.bass_utils import run_bass_kernel_spmd

    B, S, _ = x.shape
    T = B * S
    x2d = np.ascontiguousarray(x, dtype=np.float32).reshape(T, H)

    idx_e, cw_e = _route(x2d, np.asarray(router_w, dtype=np.float32))
    loads = [len(i) for i in idx_e]
    plan = _plan_overflow(loads)
    use_b2 = bool(np.any(b2))

    if plan is not None:
        Q, V, slices = plan
        blocks, cap = _blocks_for(Q)
        blocks.append((Q, V))
        cap = Q + V
        split = True
    else:
        blocks, cap = _blocks_for(max(loads))
        slices = None
        split = False

    key = (tuple(blocks), use_b2, split)
    nc = _kernel_cache.get(key)
    if nc is None:
        nc = _build(blocks, use_b2, split)
        _kernel_cache[key] = nc

    packs = {
        e: np.ascontiguousarray(np.concatenate(
            [_pack_w12(w1[e]), _pack_w12(w2[e]), _pack_w3(w3[e])], axis=2
        ))
        for e in range(E)
    }
    nprim = blocks[-2][0] + blocks[-2][1] if split else cap  # Q or cap
    in_maps = []
    for c in range(E):
        idx = idx_e[c][:nprim]
        xg = np.zeros((cap, H), dtype=np.float32)
        xg[: len(idx)] = x2d[idx]
        if split:
            e2, s2, l2 = slices[c]
            if l2:
                xg[nprim:nprim + l2] = x2d[idx_e[e2][s2:s2 + l2]]
        m = {
            "xT": _pack_x(xg.astype(_BF16), blocks),
            "w123": packs[c],
            "b1": np.ascontiguousarray(
                np.asarray(b1[c], dtype=np.float32).reshape(F // 128, 128).T
            ),
            "b3": np.ascontiguousarray(
                np.asarray(b3[c], dtype=np.float32).reshape(NHT, 128).T
            ),
        }
        if split:
            m["w123o"] = packs[slices[c][0]]
        if use_b2:
            m["b2"] = np.ascontiguousarray(
                np.asarray(b2[c], dtype=np.float32).reshape(F // 128, 128).T
            )
        in_maps.append(m)

    global _last_in_maps
    _last_in_maps = in_maps
    res = run_bass_kernel_spmd(nc, in_maps, core_ids=list(range(E)))

    out = np.zeros((T, H), dtype=np.float32)
    for c in range(E):
        yTe = res.results[c]["yT"]
        idx = idx_e[c][:nprim]
        n = len(idx)
        ye = _unpack_y(yTe, blocks[:-1] if split else blocks, n)
        out[idx] += ye * cw_e[c][:n, None]
        if split:
            e2, s2, l2 = slices[c]
            if l2:
                off = blocks[-1][0]
                sz = blocks[-1][1]
                blk = yTe[:, NHT * off:NHT * (off + sz)].astype(np.float32)
                yo = blk.reshape(128, NHT, sz).transpose(2, 1, 0).reshape(sz, H)
                oi = idx_e[e2][s2:s2 + l2]
                out[oi] += yo[:l2] * cw_e[e2][s2:s2 + l2, None]
    return out.reshape(B, S, H)
